# revision 1
# baseline (speedup 1.0000x reference)
"""EnhancedLoRALinear Trainium2 kernel.

Computes, for x:[4,8192,1024] and torch-style weights (out,in):
    out = x @ (W + W_res)^T + b + sigmoid(x @ W_gate^T) * (2 * (x @ W_down^T) @ W_up^T)

Strategy:
  - Data-parallel: the 32768 tokens are split across 8 NeuronCores (4096 each);
    the small weight matrices are replicated.
  - Algebraic fold: main + residual share one matmul with Wc = W + W_res.
  - Host prep: weights are pre-transposed to [in, out] so the contraction dim
    lands on SBUF partitions; x shards are pre-transposed to [in, tokens] for
    the same reason. LoRA scaling (2.0, exact in fp32) is folded into W_up^T.
  - Device: fp32r matmuls (full PE rate at moving free dim >= 256). Per
    128-token tile and 512-wide output half: a K=1 ones-row matmul seeds the
    main PSUM with the bias, 8 k-tile matmuls accumulate the main path, 8 the
    gate path, one K=16 matmul applies the LoRA up-projection from an
    [R=16, 512-token] down-projection computed once per 512 tokens. Sigmoid
    runs on ScalarE; gate*lora and +main on VectorE.
  - Sync-wait budget: fp32r matmuls can encode ONE hw sync-wait, other engine
    ops TWO. Hence: each multi-part tensor loads with a single DMA instruction
    (one queue semaphore), warm-up matmuls make the PE observe every weight
    DMA semaphore first (enforced via ordering deps), and the epilogue is
    shaped so every op joins at most two semaphores.
"""

import ml_dtypes
import numpy as np

_BF16 = ml_dtypes.bfloat16

import concourse.bass as bass
import concourse.bacc as bacc
import concourse.mybir as mybir
import concourse.tile as tile
from concourse.bass_utils import run_bass_kernel_spmd
from concourse.tile_rust import add_dep_helper

N_CORES = 8
B, S = 4, 8192
TOK = B * S                  # 32768 tokens total
T = TOK // N_CORES           # 4096 tokens per core
I = 1024                     # in_features
O = 1024                     # out_features
R = 16                       # lora rank
SCALING = 2.0                # lora_alpha / r (exact power of two)
KT = I // 128                # 8 contraction tiles
TG = 512                     # token group (down-projection batch)
NG = T // TG                 # 8 groups per core
NH = O // 512                # 2 output halves

F32 = mybir.dt.float32
F32R = mybir.dt.float32r


def _build_nc():
    nc = bacc.Bacc(None)

    xt = nc.dram_tensor("xt", [I, T], F32R, kind="ExternalInput")
    wct = nc.dram_tensor("wct", [I, O], F32R, kind="ExternalInput")
    wgt = nc.dram_tensor("wgt", [I, O], mybir.dt.bfloat16, kind="ExternalInput")
    xtb = nc.dram_tensor("xtb", [I, T], mybir.dt.bfloat16, kind="ExternalInput")
    wdt = nc.dram_tensor("wdt", [I, R], F32R, kind="ExternalInput")
    wut2 = nc.dram_tensor("wut2", [R, O], F32R, kind="ExternalInput")
    biasbc = nc.dram_tensor("biasbc", [128, O], F32, kind="ExternalInput")
    out = nc.dram_tensor("out", [T, O], F32, kind="ExternalOutput")

    # [i, o] -> [p, k, o] views so each weight loads with ONE DMA instruction
    xt_v = xt.rearrange("(k p) t -> p k t", p=128)
    xtb_v = xtb.rearrange("(k p) t -> p k t", p=128)
    wct_v = wct.rearrange("(k p) o -> p k o", p=128)
    wgt_v = wgt.rearrange("(k p) o -> p k o", p=128)
    wdt_v = wdt.rearrange("(k p) r -> p k r", p=128)

    sig = mybir.ActivationFunctionType.Sigmoid
    mult = mybir.AluOpType.mult
    add = mybir.AluOpType.add

    with tile.TileContext(nc) as tc:
        with (
            tc.tile_pool(name="wpool", bufs=1) as wpool,
            tc.tile_pool(name="xpool", bufs=3) as xpool,
            tc.tile_pool(name="opool", bufs=3) as opool,
            tc.tile_pool(name="epool", bufs=3) as epool,
            tc.tile_pool(name="psum", bufs=1, space="PSUM") as pp,
        ):
            # --- resident weights, one DMA each ---
            wc_sb = wpool.tile([128, KT, O], F32R)
            wg_sb = wpool.tile([128, KT, O], mybir.dt.bfloat16)
            wd_sb = wpool.tile([128, KT, R], F32R)
            wu_sb = wpool.tile([R, O], F32R)
            bias_bc = wpool.tile([128, O], F32)

            for k in range(KT):
                nc.sync.dma_start(out=wc_sb[:, k, :], in_=wct_v[:, k, :])
            nc.sync.dma_start(out=wg_sb[:, 0:4, :], in_=wgt_v[:, 0:4, :])
            nc.sync.dma_start(out=wg_sb[:, 4:8, :], in_=wgt_v[:, 4:8, :])
            nc.sync.dma_start(out=wd_sb[:, :, :], in_=wdt_v[:, :, :])
            nc.sync.dma_start(out=wu_sb[:, :], in_=wut2[:, :])
            nc.sync.dma_start(out=bias_bc[:, :], in_=biasbc[:, :])

            # HAM spin-up: ~60 junk matmuls keep the PE busy through the DMA
            # prologue so the clock gate opens before real compute starts
            junk = wpool.tile([128, 512], mybir.dt.bfloat16)
            nc.gpsimd.memset(junk[:, :], 0.0)
            warm = pp.tile([128, 512], F32, tag="warm")
            spin = None
            for i in range(110):
                spin = nc.tensor.matmul(warm[:, :], junk[:, 0:128], junk[:, :],
                                        start=True, stop=True)

            # warm-up matmuls: make the PE observe each weight-DMA semaphore
            # (fp32r matmuls can encode only one sync-wait downstream)
            warms = [
                nc.tensor.matmul(warm[0:1, :], wc_sb[:, k, 0:1],
                                 wc_sb[:, k, 0:512], start=True, stop=True)
                for k in range(KT)
            ] + [
                nc.tensor.matmul(warm[0:1, :], wg_sb[:, 0, 0:1],
                                 wg_sb[:, 0, 0:512], start=True, stop=True),
                nc.tensor.matmul(warm[0:1, :], wg_sb[:, 4, 0:1],
                                 wg_sb[:, 4, 0:512], start=True, stop=True),
                nc.tensor.matmul(warm[0:16, 0:16], wd_sb[:, 0, :],
                                 wd_sb[:, 0, :], start=True, stop=True),
                nc.tensor.matmul(warm[0:1, :], wu_sb[:, 0:1],
                                 wu_sb[:, 0:512], start=True, stop=True),
                spin,
            ]
            first_real = []  # first matmul of each psum group in group 0

            for g in range(NG):
                tg0 = g * TG
                xt_sb = xpool.tile([128, KT, TG], F32R, tag="xt")
                nc.sync.dma_start(
                    out=xt_sb[:, :, :], in_=xt_v[:, :, tg0 : tg0 + TG]
                )
                xtb_sb = xpool.tile([128, KT, TG], mybir.dt.bfloat16, tag="xtb")
                nc.sync.dma_start(
                    out=xtb_sb[:, :, :], in_=xtb_v[:, :, tg0 : tg0 + TG]
                )

                # LoRA down-projection for the whole 512-token group: [R, TG]
                dps = pp.tile([R, TG], F32, tag="misc")
                for k in range(KT):
                    mm = nc.tensor.matmul(
                        dps[:, :],
                        wd_sb[:, k, :],
                        xt_sb[:, k, :],
                        start=(k == 0),
                        stop=(k == KT - 1),
                    )
                    if g == 0 and k == 0:
                        first_real.append(mm)
                down_sb = epool.tile([R, TG], F32R, tag="down")
                nc.vector.tensor_copy(down_sb[:, :], dps[:, :])

                for t in range(TG // 128):
                    tsl = slice(t * 128, (t + 1) * 128)
                    out_sb = opool.tile([128, O], F32, tag="out")
                    for oh in range(NH):
                        osl = slice(oh * 512, (oh + 1) * 512)
                        mps = pp.tile([128, 512], F32, tag=f"main{oh}")
                        gps = pp.tile([128, 512], F32, tag=f"gate{oh}")
                        lps = pp.tile([128, 512], F32, tag=f"lora{oh}")
                        for k in range(KT):
                            mm = nc.tensor.matmul(
                                mps[:, :],
                                xt_sb[:, k, tsl],
                                wc_sb[:, k, osl],
                                start=(k == 0),
                                stop=(k == KT - 1),
                            )
                            if g == 0 and t == 0 and k == 0:
                                first_real.append(mm)
                        for k in range(KT):
                            nc.tensor.ldweights(xtb_sb[:, k, tsl])
                            mm = nc.tensor.matmul(
                                gps[:, :],
                                xtb_sb[:, k, tsl],
                                wg_sb[:, k, osl],
                                start=(k == 0),
                                stop=(k == KT - 1),
                            )
                            if g == 0 and t == 0 and k == 0:
                                first_real.append(mm)
                        mm = nc.tensor.matmul(
                            lps[:, :],
                            down_sb[:, tsl],
                            wu_sb[:, osl],
                            start=True,
                            stop=True,
                        )
                        if g == 0 and t == 0:
                            first_real.append(mm)
                        g_sb = epool.tile([128, 512], F32, tag="sig")
                        nc.scalar.activation(g_sb[:, :], gps[:, :], sig)
                        gl_sb = epool.tile([128, 512], F32, tag="gl")
                        nc.vector.tensor_tensor(
                            gl_sb[:, :], g_sb[:, :], lps[:, :], mult
                        )
                        nc.gpsimd.tensor_tensor(
                            gl_sb[:, :], gl_sb[:, :], bias_bc[:, osl], add
                        )
                        nc.vector.tensor_tensor(
                            out_sb[:, osl], gl_sb[:, :], mps[:, :], add
                        )
                    nc.sync.dma_start(
                        out=out[tg0 + t * 128 : tg0 + (t + 1) * 128, :],
                        in_=out_sb[:, :],
                    )

            # ordering-only deps: every warm-up precedes the first matmul of
            # each group-0 psum chain, so no real matmul lands before the PE
            # has observed all weight DMA semaphores
            for w in warms:
                for fr in first_real:
                    add_dep_helper(fr.ins, w.ins, False,
                                   "warmups before real matmuls")
    nc.compile()
    return nc


_NC_CACHE = None


def _get_nc():
    global _NC_CACHE
    if _NC_CACHE is None:
        _NC_CACHE = _build_nc()
    return _NC_CACHE


def _prep_inputs(x, W, b, W_down, W_up, W_gate, W_res):
    x = np.asarray(x, dtype=np.float32).reshape(TOK, I)
    wct = np.ascontiguousarray((np.asarray(W) + np.asarray(W_res)).T.astype(np.float32))
    wgt = np.ascontiguousarray(np.asarray(W_gate).T.astype(_BF16))
    wdt = np.ascontiguousarray(np.asarray(W_down).T.astype(np.float32))
    wut2 = np.ascontiguousarray((SCALING * np.asarray(W_up)).T.astype(np.float32))
    biasbc = np.ascontiguousarray(
        np.broadcast_to(np.asarray(b, dtype=np.float32).reshape(1, O), (128, O))
    )
    in_maps = []
    for c in range(N_CORES):
        xt_c = np.ascontiguousarray(x[c * T : (c + 1) * T, :].T)
        xtb_c = xt_c.astype(_BF16)
        in_maps.append(
            {
                "xt": xt_c,
                "xtb": xtb_c,
                "wct": wct,
                "wgt": wgt,
                "wdt": wdt,
                "wut2": wut2,
                "biasbc": biasbc,
            }
        )
    return in_maps


def run(inputs, trace=False, **kwargs):
    """Build + run on the 8 NeuronCores. Returns (full_output, BassKernelResults)."""
    nc = _get_nc()
    in_maps = _prep_inputs(**inputs)
    res = run_bass_kernel_spmd(
        nc, in_maps, list(range(N_CORES)), trace=trace, **kwargs
    )
    shards = [res.results[c]["out"] for c in range(N_CORES)]
    full = np.concatenate(shards, axis=0).reshape(B, S, O)
    return full, res


def kernel(**inputs):
    out, _ = run(inputs, trace=False)
    return out



# revision 3
# speedup vs baseline: 1.5062x; 1.5062x over previous
"""EnhancedLoRALinear Trainium2 kernel (v2: bf16 main + fp8 DoubleRow gate/down).

Computes, for x:[4,8192,1024] and torch-style weights (out,in):
    out = x @ (W + W_res)^T + b + sigmoid(x @ W_gate^T) * (2 * (x @ W_down^T) @ W_up^T)

Strategy:
  - Data-parallel: the 32768 tokens are split across 8 NeuronCores (4096 each);
    the small weight matrices are replicated.
  - Algebraic fold: main + residual share one matmul with Wc = W + W_res.
  - Precision split (rel-err budget is 2e-2; measured 3.8e-3 on host):
      * main path in bf16 (x bf16 stationary, Wc bf16 moving) - full PE rate
        with FWL weight loads.
      * gate path in fp8 e4m3 with perf_mode=DoubleRow: K=1024 contraction in
        4 matmuls of K=256 (2 k-elements per cell). W_gate is scaled x64 so
        its entries leave the fp8 subnormal range; the sigmoid applies
        scale=1/64 to undo it. Sigmoid squashes the residual quantization
        error and the gate only multiplies the small LoRA term.
      * down-projection in fp8 DoubleRow too (W_down scaled x64; the 1/64
        plus the LoRA scaling 2.0 are folded into W_up host-side).
      * lora-up in bf16, with the two 512-wide output halves packed into
        concurrent row-tiled matmuls (K=16 each, rows 0-15 and 32-47).
  - DMA: weights issue on the Sync queue while x tiles issue on the Scalar
    queue (each dma_start costs ~1us of issue time on its engine; the v1
    kernel serialized everything on Sync which delayed the first real matmul
    to ~30us). Output tiles go back on Sync.
  - A short junk-matmul spin keeps the PE busy through the DMA prologue so
    the HAM clock gate is open when real work starts; ordering-only deps pin
    every junk matmul before the first matmul of each PSUM group.
"""

import ml_dtypes
import numpy as np

_BF16 = ml_dtypes.bfloat16
_F8E4 = ml_dtypes.float8_e4m3  # IEEE e4m3 (bias 7, max 240) == TRN FP8_EXP4

import concourse.bass as bass
import concourse.bacc as bacc
import concourse.mybir as mybir
import concourse.tile as tile
from concourse.bass_utils import run_bass_kernel_spmd
from concourse.tile_rust import add_dep_helper

N_CORES = 8
B, S = 4, 8192
TOK = B * S                  # 32768 tokens total
T = TOK // N_CORES           # 4096 tokens per core
I = 1024                     # in_features
O = 1024                     # out_features
R = 16                       # lora rank
SCALING = 2.0                # lora_alpha / r
KT = I // 128                # 8 bf16 contraction tiles
KJ = I // 256                # 4 fp8 DoubleRow contraction tiles
TG = 512                     # token group (down-projection batch)
NG = T // TG                 # 8 groups per core
NH = O // 512                # 2 output halves
WS = 64.0                    # fp8 weight scale (exact power of two)
N_JUNK = 48                  # HAM warm-up matmuls (cover the DMA prologue)

F32 = mybir.dt.float32
BF16 = mybir.dt.bfloat16
F8 = mybir.dt.float8e4
DR = mybir.MatmulPerfMode.DoubleRow


def _build_nc():
    nc = bacc.Bacc(None)

    # Pre-swizzled DRAM layouts (one contiguous DMA each):
    #   bf16 x:   [128p, 8k, T]   k = kt*128 + p
    #   fp8  x:   [128p, 4j, 2i, T]  k = j*256 + i*128 + p
    xtb = nc.dram_tensor("xtb", [128, KT, T], BF16, kind="ExternalInput")
    x8 = nc.dram_tensor("x8", [128, KJ, 2, T], F8, kind="ExternalInput")
    wct = nc.dram_tensor("wct", [128, KT, O], BF16, kind="ExternalInput")
    wg8 = nc.dram_tensor("wg8", [128, KJ, 2, O], F8, kind="ExternalInput")
    wd8 = nc.dram_tensor("wd8", [128, KJ, 2, R], F8, kind="ExternalInput")
    wut2 = nc.dram_tensor("wut2", [48, O], BF16, kind="ExternalInput")
    biasbc = nc.dram_tensor("biasbc", [128, O], F32, kind="ExternalInput")
    out = nc.dram_tensor("out", [T, O], F32, kind="ExternalOutput")

    sig = mybir.ActivationFunctionType.Sigmoid
    mult = mybir.AluOpType.mult
    add = mybir.AluOpType.add

    with tile.TileContext(nc) as tc:
        with (
            tc.tile_pool(name="wpool", bufs=1) as wpool,
            tc.tile_pool(name="xpool", bufs=3) as xpool,
            tc.tile_pool(name="opool", bufs=3) as opool,
            tc.tile_pool(name="epool", bufs=3) as epool,
            tc.tile_pool(name="psum", bufs=1, space="PSUM") as pp,
        ):
            # --- resident weights, one DMA each, on the Sync queue ---
            wc_sb = wpool.tile([128, KT, O], BF16)
            wg_sb = wpool.tile([128, KJ, 2, O], F8)
            wd_sb = wpool.tile([128, KJ, 2, R], F8)
            wu_sb = wpool.tile([48, O], BF16)
            bias_bc = wpool.tile([128, O], F32)

            nc.sync.dma_start(out=wc_sb[:, :, :], in_=wct[:, :, :])
            nc.sync.dma_start(out=wg_sb[:, :, :, :], in_=wg8[:, :, :, :])
            nc.sync.dma_start(out=wd_sb[:, :, :, :], in_=wd8[:, :, :, :])
            nc.sync.dma_start(out=wu_sb[:, :], in_=wut2[:, :])
            nc.sync.dma_start(out=bias_bc[:, :], in_=biasbc[:, :])

            # HAM spin-up: junk matmuls keep the PE busy through the DMA
            # prologue so the clock gate is open before real compute starts
            junk = wpool.tile([128, 512], BF16)
            nc.gpsimd.memset(junk[:, :], 0.0)
            warm = pp.tile([128, 512], F32, tag="warm")
            spin = None
            for _ in range(N_JUNK):
                spin = nc.tensor.matmul(warm[:, :], junk[:, 0:128], junk[:, :],
                                        start=True, stop=True)
            first_real = []  # first matmul of each psum tag's first group

            for g in range(NG):
                tg0 = g * TG
                xt_sb = xpool.tile([128, KT, TG], BF16, tag="xtb")
                nc.scalar.dma_start(
                    out=xt_sb[:, :, :], in_=xtb[:, :, tg0 : tg0 + TG]
                )
                x8_sb = xpool.tile([128, KJ, 2, TG], F8, tag="x8")
                nc.scalar.dma_start(
                    out=x8_sb[:, :, :, :], in_=x8[:, :, :, tg0 : tg0 + TG]
                )

                # LoRA down-projection (64x scaled) for the whole group: [R, TG]
                dps = pp.tile([R, TG], F32, tag="dps")
                for j in range(KJ):
                    mm = nc.tensor.matmul(
                        dps[:, :],
                        wd_sb[:, j, :, :],
                        x8_sb[:, j, :, :],
                        start=(j == 0),
                        stop=(j == KJ - 1),
                        perf_mode=DR,
                    )
                    if g == 0 and j == 0:
                        first_real.append(mm)
                # duplicate down rows at partitions 0-15 and 32-47 so the two
                # output halves' lora-up matmuls can run row-tiled concurrently
                down_sb = epool.tile([48, TG], BF16, tag="down")
                nc.vector.tensor_copy(down_sb[0:R, :], dps[:, :])
                nc.vector.tensor_copy(down_sb[32 : 32 + R, :], dps[:, :])

                for t in range(TG // 128):
                    tsl = slice(t * 128, (t + 1) * 128)
                    out_sb = opool.tile([128, O], F32, tag="out")
                    lset = []
                    for oh in range(NH):
                        osl = slice(oh * 512, (oh + 1) * 512)
                        mps = pp.tile([128, 512], F32, tag=f"main{oh}")
                        gps = pp.tile([128, 512], F32, tag=f"gate{oh}")
                        for k in range(KT):
                            mm = nc.tensor.matmul(
                                mps[:, :],
                                xt_sb[:, k, tsl],
                                wc_sb[:, k, osl],
                                start=(k == 0),
                                stop=(k == KT - 1),
                            )
                            if g == 0 and t == 0 and k == 0:
                                first_real.append(mm)
                        for j in range(KJ):
                            mm = nc.tensor.matmul(
                                gps[:, :],
                                x8_sb[:, j, :, tsl],
                                wg_sb[:, j, :, osl],
                                start=(j == 0),
                                stop=(j == KJ - 1),
                                perf_mode=DR,
                            )
                            if g == 0 and t == 0 and j == 0:
                                first_real.append(mm)
                        lset.append((oh, osl, mps, gps))
                    # the two lora-up matmuls run concurrently in row groups
                    # 0 (rows 0-15) and 1 (rows 32-47)
                    lps = {}
                    for oh, osl, _, _ in lset:
                        r0 = 32 * oh
                        lp_t = pp.tile([128, 512], F32, tag=f"lora{oh}")
                        lps[oh] = lp_t
                        mm = nc.tensor.matmul(
                            lps[oh][:, :],
                            down_sb[r0 : r0 + R, tsl],
                            wu_sb[r0 : r0 + R, osl],
                            start=True,
                            stop=True,
                        )
                        if g == 0 and t == 0:
                            first_real.append(mm)
                    for oh, osl, mps, gps in lset:
                        g_sb = epool.tile([128, 512], F32, tag=f"sig{oh}")
                        nc.scalar.activation(g_sb[:, :], gps[:, :], sig,
                                             scale=1.0 / WS)
                        gl_sb = epool.tile([128, 512], F32, tag=f"gl{oh}")
                        nc.vector.tensor_tensor(
                            gl_sb[:, :], g_sb[:, :], lps[oh][:, :], mult
                        )
                        nc.gpsimd.tensor_tensor(
                            gl_sb[:, :], gl_sb[:, :], bias_bc[:, osl], add
                        )
                        nc.vector.tensor_tensor(
                            out_sb[:, osl], gl_sb[:, :], mps[:, :], add
                        )
                    nc.sync.dma_start(
                        out=out[tg0 + t * 128 : tg0 + (t + 1) * 128, :],
                        in_=out_sb[:, :],
                    )

            # ordering-only deps: all junk matmuls precede the first matmul of
            # each psum chain so the PE queue never stalls behind real matmuls
            # waiting on input DMAs
            for fr in first_real:
                add_dep_helper(fr.ins, spin.ins, False,
                               "warmup before real matmuls")
    nc.compile()
    return nc


_NC_CACHE = None


def _get_nc():
    global _NC_CACHE
    if _NC_CACHE is None:
        _NC_CACHE = _build_nc()
    return _NC_CACHE


def _swizzle_k(a, pair):
    """[I, N] -> [128, KT, N] (pair=False) or [128, KJ, 2, N] (pair=True)."""
    n = a.shape[1]
    if pair:
        return np.ascontiguousarray(
            a.reshape(KJ, 2, 128, n).transpose(2, 0, 1, 3)
        )
    return np.ascontiguousarray(a.reshape(KT, 128, n).transpose(1, 0, 2))


def _prep_inputs(x, W, b, W_down, W_up, W_gate, W_res):
    x = np.asarray(x, dtype=np.float32).reshape(TOK, I)
    wct = _swizzle_k(
        (np.asarray(W) + np.asarray(W_res)).T.astype(_BF16), pair=False
    )
    wg8 = _swizzle_k(
        (WS * np.asarray(W_gate)).T.astype(_F8E4), pair=True
    )
    wd8 = _swizzle_k((WS * np.asarray(W_down)).T.astype(_F8E4), pair=True)
    # lora-up weights: scaling/WS folded in; rows duplicated at 0-15 and 32-47
    wu = (SCALING / WS * np.asarray(W_up)).T.astype(_BF16)  # [R, O]
    wut2 = np.zeros((48, O), dtype=_BF16)
    wut2[0:R] = wu
    wut2[32 : 32 + R] = wu
    biasbc = np.ascontiguousarray(
        np.broadcast_to(np.asarray(b, dtype=np.float32).reshape(1, O), (128, O))
    )
    in_maps = []
    for c in range(N_CORES):
        xt_c = x[c * T : (c + 1) * T, :].T  # [I, T]
        in_maps.append(
            {
                "xtb": _swizzle_k(xt_c.astype(_BF16), pair=False),
                "x8": _swizzle_k(xt_c.astype(_F8E4), pair=True),
                "wct": wct,
                "wg8": wg8,
                "wd8": wd8,
                "wut2": wut2,
                "biasbc": biasbc,
            }
        )
    return in_maps


def run(inputs, trace=False, **kwargs):
    """Build + run on the 8 NeuronCores. Returns (full_output, BassKernelResults)."""
    nc = _get_nc()
    in_maps = _prep_inputs(**inputs)
    res = run_bass_kernel_spmd(
        nc, in_maps, list(range(N_CORES)), trace=trace, **kwargs
    )
    shards = [res.results[c]["out"] for c in range(N_CORES)]
    full = np.concatenate(shards, axis=0).reshape(B, S, O)
    return full, res


def kernel(**inputs):
    out, _ = run(inputs, trace=False)
    return out


# revision 12
# speedup vs baseline: 1.5272x; 1.0140x over previous
"""EnhancedLoRALinear Trainium2 kernel (v2: bf16 main + fp8 DoubleRow gate/down).

Computes, for x:[4,8192,1024] and torch-style weights (out,in):
    out = x @ (W + W_res)^T + b + sigmoid(x @ W_gate^T) * (2 * (x @ W_down^T) @ W_up^T)

Strategy:
  - Data-parallel: the 32768 tokens are split across 8 NeuronCores (4096 each);
    the small weight matrices are replicated.
  - Algebraic fold: main + residual share one matmul with Wc = W + W_res.
  - Precision split (rel-err budget is 2e-2; measured 3.8e-3 on host):
      * main path in bf16 (x bf16 stationary, Wc bf16 moving) - full PE rate
        with FWL weight loads.
      * gate path in fp8 e4m3 with perf_mode=DoubleRow: K=1024 contraction in
        4 matmuls of K=256 (2 k-elements per cell). W_gate is scaled x64 so
        its entries leave the fp8 subnormal range; the sigmoid applies
        scale=1/64 to undo it. Sigmoid squashes the residual quantization
        error and the gate only multiplies the small LoRA term.
      * down-projection in fp8 DoubleRow too (W_down scaled x64; the 1/64
        plus the LoRA scaling 2.0 are folded into W_up host-side).
      * lora-up in bf16, with the two 512-wide output halves packed into
        concurrent row-tiled matmuls (K=16 each, rows 0-15 and 32-47).
  - DMA: weights issue on the Sync queue while x tiles issue on the Scalar
    queue (each dma_start costs ~1us of issue time on its engine; the v1
    kernel serialized everything on Sync which delayed the first real matmul
    to ~30us). Output tiles go back on Sync.
  - A short junk-matmul spin keeps the PE busy through the DMA prologue so
    the HAM clock gate is open when real work starts; ordering-only deps pin
    every junk matmul before the first matmul of each PSUM group.
"""

import ml_dtypes
import numpy as np

_BF16 = ml_dtypes.bfloat16
_F8E4 = ml_dtypes.float8_e4m3  # IEEE e4m3 (bias 7, max 240) == TRN FP8_EXP4

import concourse.bass as bass
import concourse.bacc as bacc
import concourse.mybir as mybir
import concourse.tile as tile
from concourse.bass_utils import run_bass_kernel_spmd
from concourse.tile_rust import add_dep_helper

N_CORES = 8
B, S = 4, 8192
TOK = B * S                  # 32768 tokens total
T = TOK // N_CORES           # 4096 tokens per core
I = 1024                     # in_features
O = 1024                     # out_features
R = 16                       # lora rank
SCALING = 2.0                # lora_alpha / r
KT = I // 128                # 8 bf16 contraction tiles
KJ = I // 256                # 4 fp8 DoubleRow contraction tiles
TG = 512                     # token group (down-projection batch)
NG = T // TG                 # 8 groups per core
NH = O // 512                # 2 output halves
WS = 64.0                    # fp8 weight scale (exact power of two)
N_JUNK = 26                  # HAM warm-up matmuls (cover the DMA prologue)

F32 = mybir.dt.float32
BF16 = mybir.dt.bfloat16
F8 = mybir.dt.float8e4
DR = mybir.MatmulPerfMode.DoubleRow


def _build_nc():
    nc = bacc.Bacc(None)

    # Pre-swizzled DRAM layouts (one contiguous DMA each):
    #   bf16 x:   [128p, 8k, T]   k = kt*128 + p
    #   fp8  x:   [128p, 4j, 2i, T]  k = j*256 + i*128 + p
    xtb = nc.dram_tensor("xtb", [128, KT, T], BF16, kind="ExternalInput")
    x8 = nc.dram_tensor("x8", [128, KJ, 2, T], F8, kind="ExternalInput")
    wct = nc.dram_tensor("wct", [128, KT, O], BF16, kind="ExternalInput")
    wg8 = nc.dram_tensor("wg8", [128, KJ, 2, O], F8, kind="ExternalInput")
    wd8 = nc.dram_tensor("wd8", [128, KJ, 2, R], F8, kind="ExternalInput")
    wut2 = nc.dram_tensor("wut2", [48, O], BF16, kind="ExternalInput")
    biasr = nc.dram_tensor("biasr", [1, O], F32, kind="ExternalInput")
    out = nc.dram_tensor("out", [T, O], F32, kind="ExternalOutput")

    sig = mybir.ActivationFunctionType.Sigmoid
    mult = mybir.AluOpType.mult
    add = mybir.AluOpType.add

    with tile.TileContext(nc) as tc:
        with (
            tc.tile_pool(name="wpool", bufs=1) as wpool,
            tc.tile_pool(name="xpool", bufs=3) as xpool,
            tc.tile_pool(name="opool", bufs=3) as opool,
            tc.tile_pool(name="epool", bufs=3) as epool,
            tc.tile_pool(name="psum", bufs=1, space="PSUM") as pp,
        ):
            # --- resident weights on the Sync queue, ordered by first use:
            # down weights, first wc half, gate weights, second wc half ---
            wc_sb = wpool.tile([128, KT, O], BF16)
            wg_sb = wpool.tile([128, KJ, 2, O], F8)
            wd_sb = wpool.tile([128, KJ, 2, R], F8)
            wu_sb = wpool.tile([48, O], BF16)
            bias_r = wpool.tile([1, O], F32)
            bias_bc = wpool.tile([128, O], F32)

            nc.sync.dma_start(out=wd_sb[:, :, :, :], in_=wd8[:, :, :, :])
            nc.sync.dma_start(out=wc_sb[:, 0:4, :], in_=wct[:, 0:4, :])
            nc.sync.dma_start(out=wg_sb[:, :, :, :], in_=wg8[:, :, :, :])
            nc.sync.dma_start(out=wc_sb[:, 4:8, :], in_=wct[:, 4:8, :])
            nc.sync.dma_start(out=wu_sb[:, :], in_=wut2[:, :])
            nc.sync.dma_start(out=bias_r[:, :], in_=biasr[:, :])
            nc.gpsimd.partition_broadcast(bias_bc[:, :], bias_r[0:1, :])

            # HAM spin-up: junk matmuls keep the PE busy through the DMA
            # prologue so the clock gate is open before real compute starts
            junk = wpool.tile([128, 512], BF16)
            nc.gpsimd.memset(junk[:, :], 0.0)
            warm = pp.tile([128, 512], F32, tag="warm")
            spin = None
            for _ in range(N_JUNK):
                spin = nc.tensor.matmul(warm[:, :], junk[:, 0:128], junk[:, :],
                                        start=True, stop=True)
            first_real = []  # first matmul of each psum tag's first group

            # x-tile DMAs issue on the Scalar queue, one group ahead of use
            x_tiles = {}

            def issue_x(g):
                tg0 = g * TG
                xt_t = xpool.tile([128, KT, TG], BF16, tag="xtb",
                                  name=f"xtb{g}")
                nc.scalar.dma_start(
                    out=xt_t[:, :, :], in_=xtb[:, :, tg0 : tg0 + TG]
                )
                x8_t = xpool.tile([128, KJ, 2, TG], F8, tag="x8",
                                  name=f"x8_{g}")
                nc.scalar.dma_start(
                    out=x8_t[:, :, :, :], in_=x8[:, :, :, tg0 : tg0 + TG]
                )
                x_tiles[g] = (xt_t, x8_t)

            issue_x(0)
            issue_x(1)
            for g in range(NG):
                tg0 = g * TG
                if g + 2 < NG:
                    issue_x(g + 2)
                xt_sb, x8_sb = x_tiles.pop(g)

                # LoRA down-projection (64x scaled) for the whole group: [R, TG]
                dps = pp.tile([R, TG], F32, tag="dps")
                for j in range(KJ):
                    mm = nc.tensor.matmul(
                        dps[:, :],
                        wd_sb[:, j, :, :],
                        x8_sb[:, j, :, :],
                        start=(j == 0),
                        stop=(j == KJ - 1),
                        perf_mode=DR,
                    )
                    if g == 0 and j == 0:
                        first_real.append(mm)
                # duplicate down rows at partitions 0-15 and 32-47 so the two
                # output halves' lora-up matmuls can run row-tiled concurrently
                down_sb = epool.tile([48, TG], BF16, tag="down")
                nc.vector.tensor_copy(down_sb[0:R, :], dps[:, :])
                nc.vector.tensor_copy(down_sb[32 : 32 + R, :], dps[:, :])

                for t in range(TG // 128):
                    tsl = slice(t * 128, (t + 1) * 128)
                    out_sb = opool.tile([128, O], F32, tag="out")
                    lset = []
                    for oh in range(NH):
                        osl = slice(oh * 512, (oh + 1) * 512)
                        mps = pp.tile([128, 512], F32, tag=f"main{oh}")
                        gps = pp.tile([128, 512], F32, tag=f"gate{oh}")
                        for k in range(KT):
                            mm = nc.tensor.matmul(
                                mps[:, :],
                                xt_sb[:, k, tsl],
                                wc_sb[:, k, osl],
                                start=(k == 0),
                                stop=(k == KT - 1),
                            )
                            if g == 0 and t == 0 and k == 0:
                                first_real.append(mm)
                        for j in range(KJ):
                            mm = nc.tensor.matmul(
                                gps[:, :],
                                x8_sb[:, j, :, tsl],
                                wg_sb[:, j, :, osl],
                                start=(j == 0),
                                stop=(j == KJ - 1),
                                perf_mode=DR,
                            )
                            if g == 0 and t == 0 and j == 0:
                                first_real.append(mm)
                        # DVE drains the main psum as soon as it is complete
                        # (adds the bias); frees the bank for the next tile
                        mb_sb = epool.tile([128, 512], F32, tag=f"mb{oh}")
                        nc.vector.tensor_tensor(
                            mb_sb[:, :], mps[:, :], bias_bc[:, osl], add
                        )
                        lset.append((oh, osl, mb_sb, gps))
                    # the two lora-up matmuls run concurrently in row groups
                    # 0 (rows 0-15) and 1 (rows 32-47)
                    lps = {}
                    for oh, osl, _, _ in lset:
                        r0 = 32 * oh
                        lp_t = pp.tile([128, 512], F32, tag=f"lora{oh}")
                        lps[oh] = lp_t
                        mm = nc.tensor.matmul(
                            lps[oh][:, :],
                            down_sb[r0 : r0 + R, tsl],
                            wu_sb[r0 : r0 + R, osl],
                            start=True,
                            stop=True,
                        )
                        if g == 0 and t == 0:
                            first_real.append(mm)
                    for oh, osl, mb_sb, gps in lset:
                        # sigmoid frees the gate psum; DVE mult frees the
                        # lora psum; gpsimd does the all-SBUF final add
                        g_sb = epool.tile([128, 512], F32, tag=f"sig{oh}")
                        nc.scalar.activation(g_sb[:, :], gps[:, :], sig,
                                             scale=1.0 / WS)
                        gl_sb = epool.tile([128, 512], F32, tag=f"gl{oh}")
                        nc.vector.tensor_tensor(
                            gl_sb[:, :], g_sb[:, :], lps[oh][:, :], mult
                        )
                        nc.gpsimd.tensor_tensor(
                            out_sb[:, osl], gl_sb[:, :], mb_sb[:, :], add
                        )
                    nc.sync.dma_start(
                        out=out[tg0 + t * 128 : tg0 + (t + 1) * 128, :],
                        in_=out_sb[:, :],
                    )

            # ordering-only deps: all junk matmuls precede the first matmul of
            # each psum chain so the PE queue never stalls behind real matmuls
            # waiting on input DMAs
            for fr in first_real:
                add_dep_helper(fr.ins, spin.ins, False,
                               "warmup before real matmuls")
    nc.compile()
    return nc


_NC_CACHE = None


def _get_nc():
    global _NC_CACHE
    if _NC_CACHE is None:
        _NC_CACHE = _build_nc()
    return _NC_CACHE


def _swizzle_k(a, pair):
    """[I, N] -> [128, KT, N] (pair=False) or [128, KJ, 2, N] (pair=True)."""
    n = a.shape[1]
    if pair:
        return np.ascontiguousarray(
            a.reshape(KJ, 2, 128, n).transpose(2, 0, 1, 3)
        )
    return np.ascontiguousarray(a.reshape(KT, 128, n).transpose(1, 0, 2))


def _prep_inputs(x, W, b, W_down, W_up, W_gate, W_res):
    x = np.asarray(x, dtype=np.float32).reshape(TOK, I)
    wct = _swizzle_k(
        (np.asarray(W) + np.asarray(W_res)).T.astype(_BF16), pair=False
    )
    wg8 = _swizzle_k(
        (WS * np.asarray(W_gate)).T.astype(_F8E4), pair=True
    )
    wd8 = _swizzle_k((WS * np.asarray(W_down)).T.astype(_F8E4), pair=True)
    # lora-up weights: scaling/WS folded in; rows duplicated at 0-15 and 32-47
    wu = (SCALING / WS * np.asarray(W_up)).T.astype(_BF16)  # [R, O]
    wut2 = np.zeros((48, O), dtype=_BF16)
    wut2[0:R] = wu
    wut2[32 : 32 + R] = wu
    biasr = np.ascontiguousarray(np.asarray(b, dtype=np.float32).reshape(1, O))
    in_maps = []
    for c in range(N_CORES):
        xt_c = x[c * T : (c + 1) * T, :].T  # [I, T]
        in_maps.append(
            {
                "xtb": _swizzle_k(xt_c.astype(_BF16), pair=False),
                "x8": _swizzle_k(xt_c.astype(_F8E4), pair=True),
                "wct": wct,
                "wg8": wg8,
                "wd8": wd8,
                "wut2": wut2,
                "biasr": biasr,
            }
        )
    return in_maps


def run(inputs, trace=False, **kwargs):
    """Build + run on the 8 NeuronCores. Returns (full_output, BassKernelResults)."""
    nc = _get_nc()
    in_maps = _prep_inputs(**inputs)
    res = run_bass_kernel_spmd(
        nc, in_maps, list(range(N_CORES)), trace=trace, **kwargs
    )
    shards = [res.results[c]["out"] for c in range(N_CORES)]
    full = np.concatenate(shards, axis=0).reshape(B, S, O)
    return full, res


def kernel(**inputs):
    out, _ = run(inputs, trace=False)
    return out


# revision 16
# speedup vs baseline: 1.5530x; 1.0169x over previous
"""EnhancedLoRALinear Trainium2 kernel (v2: bf16 main + fp8 DoubleRow gate/down).

Computes, for x:[4,8192,1024] and torch-style weights (out,in):
    out = x @ (W + W_res)^T + b + sigmoid(x @ W_gate^T) * (2 * (x @ W_down^T) @ W_up^T)

Strategy:
  - Data-parallel: the 32768 tokens are split across 8 NeuronCores (4096 each);
    the small weight matrices are replicated.
  - Algebraic fold: main + residual share one matmul with Wc = W + W_res.
  - Precision split (rel-err budget is 2e-2; measured 3.8e-3 on host):
      * main path in bf16 (x bf16 stationary, Wc bf16 moving) - full PE rate
        with FWL weight loads.
      * gate path in fp8 e4m3 with perf_mode=DoubleRow: K=1024 contraction in
        4 matmuls of K=256 (2 k-elements per cell). W_gate is scaled x64 so
        its entries leave the fp8 subnormal range; the sigmoid applies
        scale=1/64 to undo it. Sigmoid squashes the residual quantization
        error and the gate only multiplies the small LoRA term.
      * down-projection in fp8 DoubleRow too (W_down scaled x64; the 1/64
        plus the LoRA scaling 2.0 are folded into W_up host-side).
      * lora-up in bf16, with the two 512-wide output halves packed into
        concurrent row-tiled matmuls (K=16 each, rows 0-15 and 32-47).
  - DMA: weights issue on the Sync queue while x tiles issue on the Scalar
    queue (each dma_start costs ~1us of issue time on its engine; the v1
    kernel serialized everything on Sync which delayed the first real matmul
    to ~30us). Output tiles go back on Sync.
  - A short junk-matmul spin keeps the PE busy through the DMA prologue so
    the HAM clock gate is open when real work starts; ordering-only deps pin
    every junk matmul before the first matmul of each PSUM group.
"""

import ml_dtypes
import numpy as np

_BF16 = ml_dtypes.bfloat16
_F8E4 = ml_dtypes.float8_e4m3  # IEEE e4m3 (bias 7, max 240) == TRN FP8_EXP4

import concourse.bass as bass
import concourse.bacc as bacc
import concourse.mybir as mybir
import concourse.tile as tile
from concourse.bass_utils import run_bass_kernel_spmd
from concourse.tile_rust import add_dep_helper

N_CORES = 8
B, S = 4, 8192
TOK = B * S                  # 32768 tokens total
T = TOK // N_CORES           # 4096 tokens per core
I = 1024                     # in_features
O = 1024                     # out_features
R = 16                       # lora rank
SCALING = 2.0                # lora_alpha / r
KT = I // 128                # 8 bf16 contraction tiles
KJ = I // 256                # 4 fp8 DoubleRow contraction tiles
TG = 512                     # token group (down-projection batch)
NG = T // TG                 # 8 groups per core
NH = O // 512                # 2 output halves
WS = 64.0                    # fp8 weight scale (exact power of two)
N_JUNK = 30                  # HAM warm-up matmuls (cover the DMA prologue)

F32 = mybir.dt.float32
BF16 = mybir.dt.bfloat16
F8 = mybir.dt.float8e4
DR = mybir.MatmulPerfMode.DoubleRow


def _build_nc():
    nc = bacc.Bacc(None)

    # Pre-swizzled DRAM layouts (one contiguous DMA each):
    #   bf16 x:   [128p, 8k, T]   k = kt*128 + p
    #   fp8  x:   [128p, 4j, 2i, T]  k = j*256 + i*128 + p
    xtb = nc.dram_tensor("xtb", [128, KT, T], BF16, kind="ExternalInput")
    x8 = nc.dram_tensor("x8", [128, KJ, 2, T], F8, kind="ExternalInput")
    wct = nc.dram_tensor("wct", [128, KT, O], BF16, kind="ExternalInput")
    wg8 = nc.dram_tensor("wg8", [128, KJ, 2, O], F8, kind="ExternalInput")
    wd8 = nc.dram_tensor("wd8", [128, KJ, 2, R], F8, kind="ExternalInput")
    wut2 = nc.dram_tensor("wut2", [48, O], BF16, kind="ExternalInput")
    biasr = nc.dram_tensor("biasr", [1, O], F32, kind="ExternalInput")
    out = nc.dram_tensor("out", [T, O], F32, kind="ExternalOutput")

    sig = mybir.ActivationFunctionType.Sigmoid
    mult = mybir.AluOpType.mult
    add = mybir.AluOpType.add

    with tile.TileContext(nc) as tc:
        with (
            tc.tile_pool(name="wpool", bufs=1) as wpool,
            tc.tile_pool(name="xpool", bufs=3) as xpool,
            tc.tile_pool(name="opool", bufs=3) as opool,
            tc.tile_pool(name="epool", bufs=3) as epool,
            tc.tile_pool(name="psum", bufs=1, space="PSUM") as pp,
        ):
            # --- resident weights on the Sync queue, ordered by first use:
            # down weights, first wc half, gate weights, second wc half ---
            wc_sb = wpool.tile([128, KT, O], BF16)
            wg_sb = wpool.tile([128, KJ, 2, O], F8)
            wd_sb = wpool.tile([128, KJ, 2, R], F8)
            wu_sb = wpool.tile([48, O], BF16)
            bias_r = wpool.tile([1, O], F32)
            bias_bc = wpool.tile([128, O], F32)

            # split by output-column half, ordered by first use: the t0/oh0
            # matmuls need only the first halves of wc and wg
            nc.sync.dma_start(out=wd_sb[:, :, :, :], in_=wd8[:, :, :, :])
            nc.sync.dma_start(out=wc_sb[:, :, 0:512], in_=wct[:, :, 0:512])
            nc.sync.dma_start(out=wg_sb[:, :, :, 0:512], in_=wg8[:, :, :, 0:512])
            nc.sync.dma_start(out=wu_sb[:, :], in_=wut2[:, :])
            nc.sync.dma_start(out=bias_r[:, :], in_=biasr[:, :])
            nc.sync.dma_start(out=wc_sb[:, :, 512:1024], in_=wct[:, :, 512:1024])
            nc.sync.dma_start(out=wg_sb[:, :, :, 512:1024],
                              in_=wg8[:, :, :, 512:1024])
            nc.gpsimd.partition_broadcast(bias_bc[:, :], bias_r[0:1, :])

            # HAM spin-up: junk matmuls keep the PE busy through the DMA
            # prologue so the clock gate is open before real compute starts
            junk = wpool.tile([128, 512], BF16)
            nc.gpsimd.memset(junk[:, :], 0.0)
            warm = pp.tile([128, 512], F32, tag="warm")
            spin = None
            for _ in range(N_JUNK):
                spin = nc.tensor.matmul(warm[:, :], junk[:, 0:128], junk[:, :],
                                        start=True, stop=True)
            first_real = []  # first matmul of each psum tag's first group

            # x-tile DMAs issue on the Scalar queue, one group ahead of use
            x_tiles = {}

            def issue_x(g):
                tg0 = g * TG
                x8_t = xpool.tile([128, KJ, 2, TG], F8, tag="x8",
                                  name=f"x8_{g}")
                nc.scalar.dma_start(
                    out=x8_t[:, :, :, :], in_=x8[:, :, :, tg0 : tg0 + TG]
                )
                xt_t = xpool.tile([128, KT, TG], BF16, tag="xtb",
                                  name=f"xtb{g}")
                nc.scalar.dma_start(
                    out=xt_t[:, :, :], in_=xtb[:, :, tg0 : tg0 + TG]
                )
                x_tiles[g] = (xt_t, x8_t)

            issue_x(0)
            issue_x(1)
            for g in range(NG):
                tg0 = g * TG
                xt_sb, x8_sb = x_tiles.pop(g)

                # LoRA down-projection (64x scaled) for the whole group: [R, TG]
                dps = pp.tile([R, TG], F32, tag="dps")
                for j in range(KJ):
                    mm = nc.tensor.matmul(
                        dps[:, :],
                        wd_sb[:, j, :, :],
                        x8_sb[:, j, :, :],
                        start=(j == 0),
                        stop=(j == KJ - 1),
                        perf_mode=DR,
                    )
                    if g == 0 and j == 0:
                        first_real.append(mm)
                # duplicate down rows at partitions 0-15 and 32-47 so the two
                # output halves' lora-up matmuls can run row-tiled concurrently
                down_sb = epool.tile([48, TG], BF16, tag="down")
                nc.vector.tensor_copy(down_sb[0:R, :], dps[:, :])
                nc.vector.tensor_copy(down_sb[32 : 32 + R, :], dps[:, :])

                for t in range(TG // 128):
                    # prefetch the next-next group's x mid-group, after the
                    # DGE queue has drained (issuing early blocks the scalar
                    # engine on descriptor backpressure, delaying sigmoids)
                    if t == 2 and g + 2 < NG:
                        issue_x(g + 2)
                    tsl = slice(t * 128, (t + 1) * 128)
                    out_sb = opool.tile([128, O], F32, tag="out")
                    lset = []
                    for oh in range(NH):
                        osl = slice(oh * 512, (oh + 1) * 512)
                        mps = pp.tile([128, 512], F32, tag=f"main{oh}")
                        gps = pp.tile([128, 512], F32, tag=f"gate{oh}")
                        for k in range(KT):
                            mm = nc.tensor.matmul(
                                mps[:, :],
                                xt_sb[:, k, tsl],
                                wc_sb[:, k, osl],
                                start=(k == 0),
                                stop=(k == KT - 1),
                            )
                            if g == 0 and t == 0 and k == 0:
                                first_real.append(mm)
                        for j in range(KJ):
                            mm = nc.tensor.matmul(
                                gps[:, :],
                                x8_sb[:, j, :, tsl],
                                wg_sb[:, j, :, osl],
                                start=(j == 0),
                                stop=(j == KJ - 1),
                                perf_mode=DR,
                            )
                            if g == 0 and t == 0 and j == 0:
                                first_real.append(mm)
                        # DVE drains the main psum as soon as it is complete
                        # (adds the bias); frees the bank for the next tile
                        mb_sb = epool.tile([128, 512], F32, tag=f"mb{oh}")
                        nc.vector.tensor_tensor(
                            mb_sb[:, :], mps[:, :], bias_bc[:, osl], add
                        )
                        lset.append((oh, osl, mb_sb, gps))
                    # the two lora-up matmuls run concurrently in row groups
                    # 0 (rows 0-15) and 1 (rows 32-47)
                    lps = {}
                    for oh, osl, _, _ in lset:
                        r0 = 32 * oh
                        lp_t = pp.tile([128, 512], F32, tag=f"lora{oh}")
                        lps[oh] = lp_t
                        mm = nc.tensor.matmul(
                            lps[oh][:, :],
                            down_sb[r0 : r0 + R, tsl],
                            wu_sb[r0 : r0 + R, osl],
                            start=True,
                            stop=True,
                        )
                        if g == 0 and t == 0:
                            first_real.append(mm)
                    for oh, osl, mb_sb, gps in lset:
                        # sigmoid frees the gate psum; DVE mult frees the
                        # lora psum; gpsimd does the all-SBUF final add
                        g_sb = epool.tile([128, 512], F32, tag=f"sig{oh}")
                        nc.scalar.activation(g_sb[:, :], gps[:, :], sig,
                                             scale=1.0 / WS)
                        gl_sb = epool.tile([128, 512], F32, tag=f"gl{oh}")
                        nc.vector.tensor_tensor(
                            gl_sb[:, :], g_sb[:, :], lps[oh][:, :], mult
                        )
                        nc.gpsimd.tensor_tensor(
                            out_sb[:, osl], gl_sb[:, :], mb_sb[:, :], add
                        )
                    nc.sync.dma_start(
                        out=out[tg0 + t * 128 : tg0 + (t + 1) * 128, :],
                        in_=out_sb[:, :],
                    )

            # ordering-only deps: all junk matmuls precede the first matmul of
            # each psum chain so the PE queue never stalls behind real matmuls
            # waiting on input DMAs
            for fr in first_real:
                add_dep_helper(fr.ins, spin.ins, False,
                               "warmup before real matmuls")
    nc.compile()
    return nc


_NC_CACHE = None


def _get_nc():
    global _NC_CACHE
    if _NC_CACHE is None:
        _NC_CACHE = _build_nc()
    return _NC_CACHE


def _swizzle_k(a, pair):
    """[I, N] -> [128, KT, N] (pair=False) or [128, KJ, 2, N] (pair=True)."""
    n = a.shape[1]
    if pair:
        return np.ascontiguousarray(
            a.reshape(KJ, 2, 128, n).transpose(2, 0, 1, 3)
        )
    return np.ascontiguousarray(a.reshape(KT, 128, n).transpose(1, 0, 2))


def _prep_inputs(x, W, b, W_down, W_up, W_gate, W_res):
    x = np.asarray(x, dtype=np.float32).reshape(TOK, I)
    wct = _swizzle_k(
        (np.asarray(W) + np.asarray(W_res)).T.astype(_BF16), pair=False
    )
    wg8 = _swizzle_k(
        (WS * np.asarray(W_gate)).T.astype(_F8E4), pair=True
    )
    wd8 = _swizzle_k((WS * np.asarray(W_down)).T.astype(_F8E4), pair=True)
    # lora-up weights: scaling/WS folded in; rows duplicated at 0-15 and 32-47
    wu = (SCALING / WS * np.asarray(W_up)).T.astype(_BF16)  # [R, O]
    wut2 = np.zeros((48, O), dtype=_BF16)
    wut2[0:R] = wu
    wut2[32 : 32 + R] = wu
    biasr = np.ascontiguousarray(np.asarray(b, dtype=np.float32).reshape(1, O))
    in_maps = []
    for c in range(N_CORES):
        xt_c = x[c * T : (c + 1) * T, :].T  # [I, T]
        in_maps.append(
            {
                "xtb": _swizzle_k(xt_c.astype(_BF16), pair=False),
                "x8": _swizzle_k(xt_c.astype(_F8E4), pair=True),
                "wct": wct,
                "wg8": wg8,
                "wd8": wd8,
                "wut2": wut2,
                "biasr": biasr,
            }
        )
    return in_maps


def run(inputs, trace=False, **kwargs):
    """Build + run on the 8 NeuronCores. Returns (full_output, BassKernelResults)."""
    nc = _get_nc()
    in_maps = _prep_inputs(**inputs)
    res = run_bass_kernel_spmd(
        nc, in_maps, list(range(N_CORES)), trace=trace, **kwargs
    )
    shards = [res.results[c]["out"] for c in range(N_CORES)]
    full = np.concatenate(shards, axis=0).reshape(B, S, O)
    return full, res


def kernel(**inputs):
    out, _ = run(inputs, trace=False)
    return out


# revision 22
# speedup vs baseline: 1.5605x; 1.0048x over previous
"""EnhancedLoRALinear Trainium2 kernel (v2: bf16 main + fp8 DoubleRow gate/down).

Computes, for x:[4,8192,1024] and torch-style weights (out,in):
    out = x @ (W + W_res)^T + b + sigmoid(x @ W_gate^T) * (2 * (x @ W_down^T) @ W_up^T)

Strategy:
  - Data-parallel: the 32768 tokens are split across 8 NeuronCores (4096 each);
    the small weight matrices are replicated.
  - Algebraic fold: main + residual share one matmul with Wc = W + W_res.
  - Precision split (rel-err budget is 2e-2; measured 3.8e-3 on host):
      * main path in bf16 (x bf16 stationary, Wc bf16 moving) - full PE rate
        with FWL weight loads.
      * gate path in fp8 e4m3 with perf_mode=DoubleRow: K=1024 contraction in
        4 matmuls of K=256 (2 k-elements per cell). W_gate is scaled x64 so
        its entries leave the fp8 subnormal range; the sigmoid applies
        scale=1/64 to undo it. Sigmoid squashes the residual quantization
        error and the gate only multiplies the small LoRA term.
      * down-projection in fp8 DoubleRow too (W_down scaled x64; the 1/64
        plus the LoRA scaling 2.0 are folded into W_up host-side).
      * lora-up in bf16, with the two 512-wide output halves packed into
        concurrent row-tiled matmuls (K=16 each, rows 0-15 and 32-47).
  - DMA: weights issue on the Sync queue while x tiles issue on the Scalar
    queue (each dma_start costs ~1us of issue time on its engine; the v1
    kernel serialized everything on Sync which delayed the first real matmul
    to ~30us). Output tiles go back on Sync.
  - A short junk-matmul spin keeps the PE busy through the DMA prologue so
    the HAM clock gate is open when real work starts; ordering-only deps pin
    every junk matmul before the first matmul of each PSUM group.
"""

import ml_dtypes
import numpy as np

_BF16 = ml_dtypes.bfloat16
_F8E4 = ml_dtypes.float8_e4m3  # IEEE e4m3 (bias 7, max 240) == TRN FP8_EXP4

import concourse.bass as bass
import concourse.bacc as bacc
import concourse.mybir as mybir
import concourse.tile as tile
from concourse.bass_utils import run_bass_kernel_spmd
from concourse.tile_rust import add_dep_helper

N_CORES = 8
B, S = 4, 8192
TOK = B * S                  # 32768 tokens total
T = TOK // N_CORES           # 4096 tokens per core
I = 1024                     # in_features
O = 1024                     # out_features
R = 16                       # lora rank
SCALING = 2.0                # lora_alpha / r
KT = I // 128                # 8 bf16 contraction tiles
KJ = I // 256                # 4 fp8 DoubleRow contraction tiles
TG = 512                     # token group (down-projection batch)
NG = T // TG                 # 8 groups per core
NH = O // 512                # 2 output halves
WS = 64.0                    # fp8 weight scale (exact power of two)
N_JUNK = 22                  # HAM warm-up matmuls (cover the DMA prologue)

F32 = mybir.dt.float32
BF16 = mybir.dt.bfloat16
F8 = mybir.dt.float8e4
DR = mybir.MatmulPerfMode.DoubleRow


def _build_nc():
    nc = bacc.Bacc(None)

    # Pre-swizzled DRAM layouts (one contiguous DMA each):
    #   bf16 x:   [128p, 8k, T]   k = kt*128 + p
    #   fp8  x:   [128p, 4j, 2i, T]  k = j*256 + i*128 + p
    xtb = nc.dram_tensor("xtb", [128, KT, T], BF16, kind="ExternalInput")
    x8 = nc.dram_tensor("x8", [128, KJ, 2, T], F8, kind="ExternalInput")
    wct = nc.dram_tensor("wct", [128, KT, O], BF16, kind="ExternalInput")
    wg8 = nc.dram_tensor("wg8", [128, KJ, 2, O], F8, kind="ExternalInput")
    wd8 = nc.dram_tensor("wd8", [128, KJ, 2, R], F8, kind="ExternalInput")
    wut2 = nc.dram_tensor("wut2", [R, O], BF16, kind="ExternalInput")
    biasr = nc.dram_tensor("biasr", [1, O], F32, kind="ExternalInput")
    out = nc.dram_tensor("out", [T, O], F32, kind="ExternalOutput")

    sig = mybir.ActivationFunctionType.Sigmoid
    mult = mybir.AluOpType.mult
    add = mybir.AluOpType.add

    with tile.TileContext(nc) as tc:
        with (
            tc.tile_pool(name="wpool", bufs=1) as wpool,
            tc.tile_pool(name="xpool", bufs=3) as xpool,
            tc.tile_pool(name="opool", bufs=3) as opool,
            tc.tile_pool(name="epool", bufs=3) as epool,
            tc.tile_pool(name="psum", bufs=1, space="PSUM") as pp,
        ):
            # --- resident weights + group-0 x, split across BOTH DMA issue
            # queues (Sync + Scalar share ~400 GB/s; each alone gets ~200),
            # ordered by first use under the mains->loras->gates tile order ---
            wc_sb = wpool.tile([128, KT, O], BF16)
            wg_sb = wpool.tile([128, KJ, 2, O], F8)
            wd_sb = wpool.tile([128, KJ, 2, R], F8)
            # lora operands are zero-padded to K=128 so the lora matmuls are
            # full-row (partial-row matmuls break LDWEIGHTS prefetching)
            wu_sb = wpool.tile([128, O], BF16)
            down_pers = wpool.tile([128, TG], BF16)
            bias_r = wpool.tile([1, O], F32)
            bias_bc = wpool.tile([128, O], F32)

            nc.gpsimd.memset(wu_sb[:, :], 0.0)
            nc.gpsimd.memset(down_pers[:, :], 0.0)

            x0_8 = xpool.tile([128, KJ, 2, TG], F8, tag="x8", name="x8_0")
            x0_b = xpool.tile([128, KT, TG], BF16, tag="xtb", name="xtb0")
            # sync queue: fp8 x first (feeds the first real matmuls), then
            # down weights, first wc half, first wg half
            nc.sync.dma_start(out=x0_8[:, :, :, :], in_=x8[:, :, :, 0:TG])
            nc.sync.dma_start(out=wd_sb[:, :, :, :], in_=wd8[:, :, :, :])
            nc.sync.dma_start(out=wc_sb[:, :, 0:512], in_=wct[:, :, 0:512])
            nc.sync.dma_start(out=wg_sb[:, :, :, 0:512], in_=wg8[:, :, :, 0:512])
            # scalar queue: small lora/bias tensors, bf16 x, second halves
            nc.scalar.dma_start(out=wu_sb[0:R, :], in_=wut2[:, :])
            nc.scalar.dma_start(out=bias_r[:, :], in_=biasr[:, :])
            nc.scalar.dma_start(out=x0_b[:, :, :], in_=xtb[:, :, 0:TG])
            nc.scalar.dma_start(out=wc_sb[:, :, 512:1024],
                                in_=wct[:, :, 512:1024])
            nc.scalar.dma_start(out=wg_sb[:, :, :, 512:1024],
                                in_=wg8[:, :, :, 512:1024])
            nc.gpsimd.partition_broadcast(bias_bc[:, :], bias_r[0:1, :])

            # HAM spin-up: junk matmuls keep the PE busy through the DMA
            # prologue so the clock gate is open before real compute starts
            junk = wpool.tile([128, 512], BF16)
            nc.gpsimd.memset(junk[:, :], 0.0)
            warm = pp.tile([128, 512], F32, tag="warm")
            spin = None
            for _ in range(N_JUNK):
                spin = nc.tensor.matmul(warm[:, :], junk[:, 0:128], junk[:, :],
                                        start=True, stop=True)
            first_real = []  # first matmul of each psum tag's first group

            # x-tile DMAs issue on the Scalar queue, one group ahead of use
            x_tiles = {}

            def issue_x(g):
                tg0 = g * TG
                x8_t = xpool.tile([128, KJ, 2, TG], F8, tag="x8",
                                  name=f"x8_{g}")
                nc.scalar.dma_start(
                    out=x8_t[:, :, :, :], in_=x8[:, :, :, tg0 : tg0 + TG]
                )
                xt_t = xpool.tile([128, KT, TG], BF16, tag="xtb",
                                  name=f"xtb{g}")
                nc.scalar.dma_start(
                    out=xt_t[:, :, :], in_=xtb[:, :, tg0 : tg0 + TG]
                )
                x_tiles[g] = (xt_t, x8_t)

            x_tiles[0] = (x0_b, x0_8)
            issue_x(1)
            for g in range(NG):
                tg0 = g * TG
                xt_sb, x8_sb = x_tiles.pop(g)

                # LoRA down-projection (64x scaled) for the whole group: [R, TG]
                dps = pp.tile([R, TG], F32, tag="dps")
                for j in range(KJ):
                    mm = nc.tensor.matmul(
                        dps[:, :],
                        wd_sb[:, j, :, :],
                        x8_sb[:, j, :, :],
                        start=(j == 0),
                        stop=(j == KJ - 1),
                        perf_mode=DR,
                    )
                    if g == 0 and j == 0:
                        first_real.append(mm)
                nc.vector.tensor_copy(down_pers[0:R, :], dps[:, :])

                for t in range(TG // 128):
                    # prefetch the next-next group's x mid-group, after the
                    # DGE queue has drained (issuing early blocks the scalar
                    # engine on descriptor backpressure, delaying sigmoids)
                    if t == 2 and g + 2 < NG:
                        issue_x(g + 2)
                    tsl = slice(t * 128, (t + 1) * 128)
                    out_sb = opool.tile([128, O], F32, tag="out")
                    # tile order: all bf16 work first (mains, loras), then
                    # the fp8 gates contiguously -- the fp8-DR weight path
                    # entry costs ~190ns, so pay it once per tile, adjacent
                    # to the next group's fp8 down-projection
                    mset = {}
                    for oh in range(NH):
                        osl = slice(oh * 512, (oh + 1) * 512)
                        mps = pp.tile([128, 512], F32, tag=f"main{oh}")
                        for k in range(KT):
                            mm = nc.tensor.matmul(
                                mps[:, :],
                                xt_sb[:, k, tsl],
                                wc_sb[:, k, osl],
                                start=(k == 0),
                                stop=(k == KT - 1),
                            )
                            if g == 0 and t == 0 and k == 0:
                                first_real.append(mm)
                        # DVE drains the main psum as soon as it is complete
                        # (adds the bias); frees the bank for the next tile
                        mb_sb = epool.tile([128, 512], F32, tag=f"mb{oh}")
                        nc.vector.tensor_tensor(
                            mb_sb[:, :], mps[:, :], bias_bc[:, osl], add
                        )
                        mset[oh] = mb_sb
                    lps = {}
                    for oh in range(NH):
                        osl = slice(oh * 512, (oh + 1) * 512)
                        lp_t = pp.tile([128, 512], F32, tag=f"lora{oh}")
                        lps[oh] = lp_t
                        mm = nc.tensor.matmul(
                            lp_t[:, :],
                            down_pers[:, tsl],
                            wu_sb[:, osl],
                            start=True,
                            stop=True,
                        )
                        if g == 0 and t == 0:
                            first_real.append(mm)
                    gset = {}
                    for oh in range(NH):
                        osl = slice(oh * 512, (oh + 1) * 512)
                        gps = pp.tile([128, 512], F32, tag=f"gate{oh}")
                        for j in range(KJ):
                            mm = nc.tensor.matmul(
                                gps[:, :],
                                x8_sb[:, j, :, tsl],
                                wg_sb[:, j, :, osl],
                                start=(j == 0),
                                stop=(j == KJ - 1),
                                perf_mode=DR,
                            )
                            if g == 0 and t == 0 and j == 0:
                                first_real.append(mm)
                        gset[oh] = gps
                    last_tile = g == NG - 1 and t == TG // 128 - 1
                    for oh in range(NH):
                        osl = slice(oh * 512, (oh + 1) * 512)
                        # sigmoid frees the gate psum; DVE mult frees the
                        # lora psum; gpsimd does the all-SBUF final add
                        # (DVE on the last tile: it is 0.6us faster and on
                        # the critical path there)
                        g_sb = epool.tile([128, 512], F32, tag=f"sig{oh}")
                        nc.scalar.activation(g_sb[:, :], gset[oh][:, :], sig,
                                             scale=1.0 / WS)
                        gl_sb = epool.tile([128, 512], F32, tag=f"gl{oh}")
                        nc.vector.tensor_tensor(
                            gl_sb[:, :], g_sb[:, :], lps[oh][:, :], mult
                        )
                        eng = nc.vector if last_tile else nc.gpsimd
                        eng.tensor_tensor(
                            out_sb[:, osl], gl_sb[:, :], mset[oh][:, :], add
                        )
                    nc.sync.dma_start(
                        out=out[tg0 + t * 128 : tg0 + (t + 1) * 128, :],
                        in_=out_sb[:, :],
                    )

            # ordering-only deps: all junk matmuls precede the first matmul of
            # each psum chain so the PE queue never stalls behind real matmuls
            # waiting on input DMAs
            for fr in first_real:
                add_dep_helper(fr.ins, spin.ins, False,
                               "warmup before real matmuls")
    nc.compile()
    return nc


_NC_CACHE = None


def _get_nc():
    global _NC_CACHE
    if _NC_CACHE is None:
        _NC_CACHE = _build_nc()
    return _NC_CACHE


def _swizzle_k(a, pair):
    """[I, N] -> [128, KT, N] (pair=False) or [128, KJ, 2, N] (pair=True)."""
    n = a.shape[1]
    if pair:
        return np.ascontiguousarray(
            a.reshape(KJ, 2, 128, n).transpose(2, 0, 1, 3)
        )
    return np.ascontiguousarray(a.reshape(KT, 128, n).transpose(1, 0, 2))


def _prep_inputs(x, W, b, W_down, W_up, W_gate, W_res):
    x = np.asarray(x, dtype=np.float32).reshape(TOK, I)
    wct = _swizzle_k(
        (np.asarray(W) + np.asarray(W_res)).T.astype(_BF16), pair=False
    )
    wg8 = _swizzle_k(
        (WS * np.asarray(W_gate)).T.astype(_F8E4), pair=True
    )
    wd8 = _swizzle_k((WS * np.asarray(W_down)).T.astype(_F8E4), pair=True)
    # lora-up weights: scaling/WS folded in (zero-padded to K=128 on device)
    wut2 = np.ascontiguousarray(
        (SCALING / WS * np.asarray(W_up)).T.astype(_BF16)
    )  # [R, O]
    biasr = np.ascontiguousarray(np.asarray(b, dtype=np.float32).reshape(1, O))
    in_maps = []
    for c in range(N_CORES):
        xt_c = x[c * T : (c + 1) * T, :].T  # [I, T]
        in_maps.append(
            {
                "xtb": _swizzle_k(xt_c.astype(_BF16), pair=False),
                "x8": _swizzle_k(xt_c.astype(_F8E4), pair=True),
                "wct": wct,
                "wg8": wg8,
                "wd8": wd8,
                "wut2": wut2,
                "biasr": biasr,
            }
        )
    return in_maps


def run(inputs, trace=False, **kwargs):
    """Build + run on the 8 NeuronCores. Returns (full_output, BassKernelResults)."""
    nc = _get_nc()
    in_maps = _prep_inputs(**inputs)
    res = run_bass_kernel_spmd(
        nc, in_maps, list(range(N_CORES)), trace=trace, **kwargs
    )
    shards = [res.results[c]["out"] for c in range(N_CORES)]
    full = np.concatenate(shards, axis=0).reshape(B, S, O)
    return full, res


def kernel(**inputs):
    out, _ = run(inputs, trace=False)
    return out


# revision 33
# speedup vs baseline: 1.5833x; 1.0146x over previous
"""EnhancedLoRALinear Trainium2 kernel (v2: bf16 main + fp8 DoubleRow gate/down).

Computes, for x:[4,8192,1024] and torch-style weights (out,in):
    out = x @ (W + W_res)^T + b + sigmoid(x @ W_gate^T) * (2 * (x @ W_down^T) @ W_up^T)

Strategy:
  - Data-parallel: the 32768 tokens are split across 8 NeuronCores (4096 each);
    the small weight matrices are replicated.
  - Algebraic fold: main + residual share one matmul with Wc = W + W_res.
  - Precision split (rel-err budget is 2e-2; measured 3.8e-3 on host):
      * main path in bf16 (x bf16 stationary, Wc bf16 moving) - full PE rate
        with FWL weight loads.
      * gate path in fp8 e4m3 with perf_mode=DoubleRow: K=1024 contraction in
        4 matmuls of K=256 (2 k-elements per cell). W_gate is scaled x64 so
        its entries leave the fp8 subnormal range; the sigmoid applies
        scale=1/64 to undo it. Sigmoid squashes the residual quantization
        error and the gate only multiplies the small LoRA term.
      * down-projection in fp8 DoubleRow too (W_down scaled x64; the 1/64
        plus the LoRA scaling 2.0 are folded into W_up host-side).
      * lora-up in bf16, with the two 512-wide output halves packed into
        concurrent row-tiled matmuls (K=16 each, rows 0-15 and 32-47).
  - DMA: weights issue on the Sync queue while x tiles issue on the Scalar
    queue (each dma_start costs ~1us of issue time on its engine; the v1
    kernel serialized everything on Sync which delayed the first real matmul
    to ~30us). Output tiles go back on Sync.
  - A short junk-matmul spin keeps the PE busy through the DMA prologue so
    the HAM clock gate is open when real work starts; ordering-only deps pin
    every junk matmul before the first matmul of each PSUM group.
"""

import ml_dtypes
import numpy as np

_BF16 = ml_dtypes.bfloat16
_F8E4 = ml_dtypes.float8_e4m3  # IEEE e4m3 (bias 7, max 240) == TRN FP8_EXP4

import concourse.bass as bass
import concourse.bacc as bacc
import concourse.mybir as mybir
import concourse.tile as tile
from concourse.bass_utils import run_bass_kernel_spmd
from concourse.tile_rust import add_dep_helper

N_CORES = 8
B, S = 4, 8192
TOK = B * S                  # 32768 tokens total
T = TOK // N_CORES           # 4096 tokens per core
I = 1024                     # in_features
O = 1024                     # out_features
R = 16                       # lora rank
SCALING = 2.0                # lora_alpha / r
KT = I // 128                # 8 bf16 contraction tiles
KJ = I // 256                # 4 fp8 DoubleRow contraction tiles
TG = 512                     # token group (down-projection batch)
NG = T // TG                 # 8 groups per core
NH = O // 512                # 2 output halves
WS = 64.0                    # fp8 weight scale (exact power of two)
N_JUNK = 14                  # HAM warm-up matmuls (cover the DMA prologue)

F32 = mybir.dt.float32
BF16 = mybir.dt.bfloat16
F8 = mybir.dt.float8e4
DR = mybir.MatmulPerfMode.DoubleRow


def _build_nc():
    nc = bacc.Bacc(None)

    # Pre-swizzled DRAM layouts, group-/half-major so every DMA moves 128
    # contiguous 4-8KB rows (small descriptors throttle the DMA queues):
    #   bf16 x:   [g, 128p, 8k, TG]     k = kt*128 + p
    #   fp8  x:   [g, 128p, 4j, 2i, TG] k = j*256 + i*128 + p
    #   wc/wg:    [half, 128p, ..., 512]
    xtb = nc.dram_tensor("xtb", [NG, 128, KT, TG], BF16, kind="ExternalInput")
    x8 = nc.dram_tensor("x8", [NG, 128, KJ, 2, TG], F8, kind="ExternalInput")
    wct = nc.dram_tensor("wct", [NH, 128, KT, 512], BF16, kind="ExternalInput")
    wg8 = nc.dram_tensor("wg8", [NH, 128, KJ, 2, 512], F8,
                         kind="ExternalInput")
    wd8 = nc.dram_tensor("wd8", [128, KJ, 2, R], F8, kind="ExternalInput")
    wut2 = nc.dram_tensor("wut2", [R, O], BF16, kind="ExternalInput")
    biasr = nc.dram_tensor("biasr", [1, O], F32, kind="ExternalInput")
    out = nc.dram_tensor("out", [T, O], F32, kind="ExternalOutput")

    sig = mybir.ActivationFunctionType.Sigmoid
    mult = mybir.AluOpType.mult
    add = mybir.AluOpType.add

    with tile.TileContext(nc) as tc:
        with (
            tc.tile_pool(name="wpool", bufs=1) as wpool,
            tc.tile_pool(name="xpool", bufs=3) as xpool,
            tc.tile_pool(name="opool", bufs=3) as opool,
            tc.tile_pool(name="epool", bufs=3) as epool,
            tc.tile_pool(name="psum", bufs=1, space="PSUM") as pp,
        ):
            # --- resident weights + group-0 x, split across BOTH DMA issue
            # queues (Sync + Scalar share ~400 GB/s; each alone gets ~200),
            # ordered by first use under the mains->loras->gates tile order.
            # wc/wg are separate per-half tiles so both DMA sides stay fully
            # contiguous (128 descriptors of 4-8KB each). ---
            wc_h = [wpool.tile([128, KT, 512], BF16, name=f"wc{h}")
                    for h in range(NH)]
            wg_h = [wpool.tile([128, KJ, 2, 512], F8, name=f"wg{h}")
                    for h in range(NH)]
            wd_sb = wpool.tile([128, KJ, 2, R], F8)
            # lora operands are zero-padded to K=128 so the lora matmuls are
            # full-row (partial-row matmuls break LDWEIGHTS prefetching)
            wu_sb = wpool.tile([128, O], BF16)
            down_pers = wpool.tile([128, TG], BF16)
            bias_r = wpool.tile([1, O], F32)
            bias_bc = wpool.tile([128, O], F32)

            x0_8 = xpool.tile([128, KJ, 2, TG], F8, tag="x8", name="x8_0")
            x0_b = xpool.tile([128, KT, TG], BF16, tag="xtb", name="xtb0")
            nc.gpsimd.memset(wu_sb[:, :], 0.0)
            nc.gpsimd.memset(down_pers[:, :], 0.0)
            # sync queue: fp8 x first (feeds the first real matmuls), then
            # down weights, first wc half, first wg half
            nc.sync.dma_start(out=x0_8[:, :, :, :], in_=x8[0, :, :, :, :])
            nc.sync.dma_start(out=wd_sb[:, :, :, :], in_=wd8[:, :, :, :])
            nc.sync.dma_start(out=wc_h[0][:, :, :], in_=wct[0, :, :, :])
            nc.sync.dma_start(out=wg_h[0][:, :, :, :], in_=wg8[0, :, :, :, :])
            # scalar queue: small lora/bias tensors, bf16 x, second halves
            nc.scalar.dma_start(out=wu_sb[0:R, :], in_=wut2[:, :])
            nc.scalar.dma_start(out=bias_r[:, :], in_=biasr[:, :])
            nc.scalar.dma_start(out=x0_b[:, :, :], in_=xtb[0, :, :, :])
            nc.scalar.dma_start(out=wc_h[1][:, :, :], in_=wct[1, :, :, :])
            nc.scalar.dma_start(out=wg_h[1][:, :, :, :], in_=wg8[1, :, :, :, :])
            nc.gpsimd.partition_broadcast(bias_bc[:, :], bias_r[0:1, :])

            # HAM spin-up: junk matmuls keep the PE busy through the DMA
            # prologue so the clock gate is open before real compute starts
            junk = wpool.tile([128, 512], BF16)
            nc.gpsimd.memset(junk[:, :], 0.0)
            warm = pp.tile([128, 512], F32, tag="warm")
            spin = None
            for _ in range(N_JUNK):
                spin = nc.tensor.matmul(warm[:, :], junk[:, 0:128], junk[:, :],
                                        start=True, stop=True)
            first_real = []  # first matmul of each psum tag's first group

            # x-tile DMAs issue on the Scalar queue, one group ahead of use
            x_tiles = {}

            def issue_x(g):
                x8_t = xpool.tile([128, KJ, 2, TG], F8, tag="x8",
                                  name=f"x8_{g}")
                nc.scalar.dma_start(
                    out=x8_t[:, :, :, :], in_=x8[g, :, :, :, :]
                )
                xt_t = xpool.tile([128, KT, TG], BF16, tag="xtb",
                                  name=f"xtb{g}")
                nc.scalar.dma_start(
                    out=xt_t[:, :, :], in_=xtb[g, :, :, :]
                )
                x_tiles[g] = (xt_t, x8_t)

            x_tiles[0] = (x0_b, x0_8)
            issue_x(1)
            for g in range(NG):
                tg0 = g * TG
                xt_sb, x8_sb = x_tiles.pop(g)

                # LoRA down-projection (64x scaled) for the whole group: [R, TG]
                dps = pp.tile([R, TG], F32, tag="dps")
                for j in range(KJ):
                    mm = nc.tensor.matmul(
                        dps[:, :],
                        wd_sb[:, j, :, :],
                        x8_sb[:, j, :, :],
                        start=(j == 0),
                        stop=(j == KJ - 1),
                        perf_mode=DR,
                    )
                    if g == 0 and j == 0:
                        first_real.append(mm)
                nc.vector.tensor_copy(down_pers[0:R, :], dps[:, :])

                for t in range(TG // 128):
                    # prefetch the next-next group's x mid-group, after the
                    # DGE queue has drained (issuing early blocks the scalar
                    # engine on descriptor backpressure, delaying sigmoids)
                    if t == 2 and g + 2 < NG:
                        issue_x(g + 2)
                    tsl = slice(t * 128, (t + 1) * 128)
                    out_sb = opool.tile([128, O], F32, tag="out")
                    # tile order: all bf16 work first (mains, loras), then
                    # the fp8 gates contiguously -- the fp8-DR weight path
                    # entry costs ~190ns, so pay it once per tile, adjacent
                    # to the next group's fp8 down-projection
                    mset = {}
                    for oh in range(NH):
                        osl = slice(oh * 512, (oh + 1) * 512)
                        mps = pp.tile([128, 512], F32, tag=f"main{oh}")
                        for k in range(KT):
                            mm = nc.tensor.matmul(
                                mps[:, :],
                                xt_sb[:, k, tsl],
                                wc_h[oh][:, k, :],
                                start=(k == 0),
                                stop=(k == KT - 1),
                            )
                            if g == 0 and t == 0 and k == 0:
                                first_real.append(mm)
                        # DVE drains the main psum as soon as it is complete
                        # (adds the bias); frees the bank for the next tile
                        mb_sb = epool.tile([128, 512], F32, tag=f"mb{oh}")
                        nc.vector.tensor_tensor(
                            mb_sb[:, :], mps[:, :], bias_bc[:, osl], add
                        )
                        mset[oh] = mb_sb
                    lps = {}
                    for oh in range(NH):
                        osl = slice(oh * 512, (oh + 1) * 512)
                        lp_t = pp.tile([128, 512], F32, tag=f"lora{oh}")
                        lps[oh] = lp_t
                        mm = nc.tensor.matmul(
                            lp_t[:, :],
                            down_pers[:, tsl],
                            wu_sb[:, osl],
                            start=True,
                            stop=True,
                        )
                        if g == 0 and t == 0:
                            first_real.append(mm)
                    gset = {}
                    for oh in range(NH):
                        osl = slice(oh * 512, (oh + 1) * 512)
                        gps = pp.tile([128, 512], F32, tag=f"gate{oh}")
                        for j in range(KJ):
                            mm = nc.tensor.matmul(
                                gps[:, :],
                                x8_sb[:, j, :, tsl],
                                wg_h[oh][:, j, :, :],
                                start=(j == 0),
                                stop=(j == KJ - 1),
                                perf_mode=DR,
                            )
                            if g == 0 and t == 0 and j == 0:
                                first_real.append(mm)
                        gset[oh] = gps
                    last_tile = g == NG - 1 and t == TG // 128 - 1
                    for oh in range(NH):
                        osl = slice(oh * 512, (oh + 1) * 512)
                        # sigmoid frees the gate psum; DVE mult frees the
                        # lora psum; gpsimd does the all-SBUF final add
                        # (DVE on the last tile: it is 0.6us faster and on
                        # the critical path there)
                        g_sb = epool.tile([128, 512], F32, tag=f"sig{oh}")
                        nc.scalar.activation(g_sb[:, :], gset[oh][:, :], sig,
                                             scale=1.0 / WS)
                        gl_sb = epool.tile([128, 512], F32, tag=f"gl{oh}")
                        nc.vector.tensor_tensor(
                            gl_sb[:, :], g_sb[:, :], lps[oh][:, :], mult
                        )
                        eng = nc.vector if last_tile else nc.gpsimd
                        eng.tensor_tensor(
                            out_sb[:, osl], gl_sb[:, :], mset[oh][:, :], add
                        )
                    nc.sync.dma_start(
                        out=out[tg0 + t * 128 : tg0 + (t + 1) * 128, :],
                        in_=out_sb[:, :],
                    )

            # ordering-only deps: all junk matmuls precede the first matmul of
            # each psum chain so the PE queue never stalls behind real matmuls
            # waiting on input DMAs
            for fr in first_real:
                add_dep_helper(fr.ins, spin.ins, False,
                               "warmup before real matmuls")
    nc.compile()
    return nc


_NC_CACHE = None


def _get_nc():
    global _NC_CACHE
    if _NC_CACHE is None:
        _NC_CACHE = _build_nc()
    return _NC_CACHE


def _prep_inputs(x, W, b, W_down, W_up, W_gate, W_res):
    x = np.asarray(x, dtype=np.float32).reshape(TOK, I)
    # weights: [I, O] -> [half, 128p, kt(/pair), 512], k = kt*128 + p
    wcT = (np.asarray(W) + np.asarray(W_res)).T.astype(_BF16)
    wct = np.ascontiguousarray(
        wcT.reshape(KT, 128, NH, 512).transpose(2, 1, 0, 3)
    )
    wgT = (WS * np.asarray(W_gate)).T.astype(_F8E4)
    wg8 = np.ascontiguousarray(
        wgT.reshape(KJ, 2, 128, NH, 512).transpose(3, 2, 0, 1, 4)
    )
    wd8 = np.ascontiguousarray(
        (WS * np.asarray(W_down)).T.astype(_F8E4)
        .reshape(KJ, 2, 128, R).transpose(2, 0, 1, 3)
    )
    # lora-up weights: scaling/WS folded in (zero-padded to K=128 on device)
    wut2 = np.ascontiguousarray(
        (SCALING / WS * np.asarray(W_up)).T.astype(_BF16)
    )  # [R, O]
    biasr = np.ascontiguousarray(np.asarray(b, dtype=np.float32).reshape(1, O))
    in_maps = []
    for c in range(N_CORES):
        xt_c = x[c * T : (c + 1) * T, :].T  # [I, T]
        # x: [I, T] -> [g, 128p, kt(/pair), TG], token t = g*TG + tau
        xtb_c = np.ascontiguousarray(
            xt_c.astype(_BF16).reshape(KT, 128, NG, TG).transpose(2, 1, 0, 3)
        )
        x8_c = np.ascontiguousarray(
            xt_c.astype(_F8E4).reshape(KJ, 2, 128, NG, TG)
            .transpose(3, 2, 0, 1, 4)
        )
        in_maps.append(
            {
                "xtb": xtb_c,
                "x8": x8_c,
                "wct": wct,
                "wg8": wg8,
                "wd8": wd8,
                "wut2": wut2,
                "biasr": biasr,
            }
        )
    return in_maps


def run(inputs, trace=False, **kwargs):
    """Build + run on the 8 NeuronCores. Returns (full_output, BassKernelResults)."""
    nc = _get_nc()
    in_maps = _prep_inputs(**inputs)
    res = run_bass_kernel_spmd(
        nc, in_maps, list(range(N_CORES)), trace=trace, **kwargs
    )
    shards = [res.results[c]["out"] for c in range(N_CORES)]
    full = np.concatenate(shards, axis=0).reshape(B, S, O)
    return full, res


def kernel(**inputs):
    out, _ = run(inputs, trace=False)
    return out
